# revision 3
# baseline (speedup 1.0000x reference)
# Expert-parallel top-1 MoE layer on 8 Trainium2 NeuronCores.
#
# Math (see reference): T=8192 tokens of dim D=1024, router picks top-1 of
# E=8 experts, token goes through that expert's MLP (D->H->D, relu), output
# scaled by the routed softmax prob.
#
# Sharding: one expert per core. The host computes the router argmax once
# (numpy) purely to decide token PLACEMENT (the "all-to-all dispatch" of the
# sharding hint): it gathers each core's tokens, transposes them to x^T and
# casts to bf16 so the device gets direct, partition-major DMA loads (no
# on-device indirect gathers or PE transposes). All VALUE math is on device:
# each core recomputes the router logits on its compacted tokens to get the
# top-1 softmax prob (= 1/sum(exp(l - max)), argmax-free), runs the expert
# MLP as two grouped GEMMs (bf16 operands, fp32 PSUM accumulation, +bias,
# relu), and scales by the prob. The host applies the inverse permutation
# (pure data movement) to assemble the full output.
import sys

sys.path.insert(0, "/opt/trn_rl_repo")

import numpy as np

T, D, H, E = 8192, 1024, 2048, 8
NCORES = 8
P = 128
CAP = 1120  # per-expert token capacity (max group this input: 1115)
BF16 = True

_cache = {}


def _blocks(cap):
    # N blocks of <=512 columns (PSUM bank width)
    nb = []
    n0 = 0
    while n0 < cap:
        w = min(512, cap - n0)
        nb.append((n0, w))
        n0 += w
    return nb


def _build(cap):
    import concourse.bass as bass
    import concourse.mybir as mybir
    import concourse.tile as tile
    from concourse import bacc

    f32 = mybir.dt.float32
    bt = mybir.dt.bfloat16 if BF16 else f32
    AL = mybir.AluOpType
    AF = mybir.ActivationFunctionType
    AX = mybir.AxisListType

    NB = _blocks(cap)
    G = (cap + P - 1) // P  # router tiles (last may be ragged; x is padded)
    GPAD = G * P  # padded token slots for the scale vector
    KD = D // P  # 8   k-chunks of D
    KH = H // P  # 16  k-chunks of H

    nc = bacc.Bacc(
        "TRN2",
        debug=False,
        enable_asserts=False,
        target_bir_lowering=False,
        num_devices=NCORES,
    )

    # x^T arrives pre-transposed/pre-gathered, one dram tensor per N block,
    # padded to a multiple of 128 columns in the last block:
    # xt{i}[p, k, j] = x_bf16[token(n0+j), k*128+p]
    xts_d = []
    for ni, (n0, nw) in enumerate(NB):
        nwp = nw if nw % P == 0 else ((nw + P - 1) // P) * P
        xts_d.append(
            nc.dram_tensor(f"xt{ni}", [P, KD, nwp], bt, kind="ExternalInput")
        )
    wr = nc.dram_tensor("wr", [P, KD, E], bt, kind="ExternalInput")
    brb_d = nc.dram_tensor("brb", [P, E], f32, kind="ExternalInput")
    # weight slabs: [m, p, k, q] so one m-slab is a single contiguous DMA
    w1t = nc.dram_tensor("w1t", [KH, P, KD, P], bt, kind="ExternalInput")
    b1t = nc.dram_tensor("b1t", [P, KH], f32, kind="ExternalInput")
    w2t = nc.dram_tensor("w2t", [KD, P, KH, P], bt, kind="ExternalInput")
    b2t = nc.dram_tensor("b2t", [P, KD], f32, kind="ExternalInput")

    yT = nc.dram_tensor("yT", [D, cap], f32, kind="ExternalOutput")

    with tile.TileContext(nc) as tc:
        with (
            tc.tile_pool(name="const", bufs=1) as cpool,
            tc.tile_pool(name="dram", bufs=1, space="DRAM") as dpool,
            tc.tile_pool(name="psum", bufs=1, space="PSUM") as pp,
            tc.tile_pool(name="main", bufs=1) as mp,
            tc.tile_pool(name="work", bufs=1) as wkp,
        ):
            # The sync DMA queue moves data strictly in emission order, so
            # emit just-in-time: router consts, then x^T block 0, then the
            # first GEMM1 slabs, then the rest interleaved by first use.
            wr_sb = cpool.tile([P, KD, E], bt, name="wr_sb")
            nc.sync.dma_start(wr_sb[:], wr.ap())
            brb = cpool.tile([P, E], f32, name="brb")
            nc.sync.dma_start(brb[:], brb_d.ap())

            xts = []
            for ni, (n0, nw) in enumerate(NB):
                nwp = xts_d[ni].shape[2]
                xsb = mp.tile([P, KD, nwp], bt, tag=f"xt{ni}", name=f"xt{ni}")
                xts.append(xsb)
            nc.sync.dma_start(xts[0][:], xts_d[0].ap())

            b1_sb = cpool.tile([P, KH], f32, name="b1_sb")
            nc.sync.dma_start(b1_sb[:], b1t.ap())

            w1s = []
            for m in range(KH):
                w1sb = cpool.tile([P, D], bt, tag=f"w1s{m}", name=f"w1sb{m}")
                w1s.append(w1sb)
            for m in range(4):
                nc.sync.dma_start(w1s[m][:], w1t.ap()[m])
            if len(NB) > 1:
                nc.sync.dma_start(xts[1][:], xts_d[1].ap())
            for m in range(4, KH):
                nc.sync.dma_start(w1s[m][:], w1t.ap()[m])

            b2_sb = cpool.tile([P, KD], f32, name="b2_sb")
            nc.sync.dma_start(b2_sb[:], b2t.ap())
            for ni in range(2, len(NB)):
                nc.sync.dma_start(xts[ni][:], xts_d[ni].ap())
            w2s = []
            for m in range(KD):
                w2sb = cpool.tile([P, H], bt, tag=f"w2s{m}", name=f"w2sb{m}")
                nc.sync.dma_start(w2sb[:], w2t.ap()[m])
                w2s.append(w2sb)

            # dummy matmuls to trip the PE HAM clock-gate to full speed while
            # the x^T DMA is still in flight
            wjunk = cpool.tile([P, 512], bt, name="wjunk")
            nc.vector.memset(wjunk[:], 0.5)
            wps = pp.tile([P, 512], f32, tag="mm0", bufs=2, name="wps")
            for w in range(10):
                nc.tensor.matmul(
                    wps[:], lhsT=wjunk[:, 0:P], rhs=wjunk[:],
                    start=(w == 0), stop=(w == 9),
                )

            scflat = dpool.tile([GPAD], f32, name="scflat")
            prq = mp.tile([P, G], f32, name="prq")
            sbc = mp.tile([P, GPAD], f32, name="sbc")
            hT = mp.tile([P, KH, cap], bt, name="hT")

            def router_tile(g):
                # top-1 softmax prob of the 128 tokens in tile g
                # (tile g spans columns [g*128, g*128+128) of the padded x^T;
                # 512-multiple N blocks keep tiles inside one block)
                ni = (g * P) // 512
                off = g * P - NB[ni][0]
                lps = pp.tile([P, E], f32, tag="small", bufs=2, name=f"lps{g}")
                for k in range(KD):
                    nc.tensor.matmul(
                        lps[:],
                        lhsT=xts[ni][:, k, off : off + P],
                        rhs=wr_sb[:, k, :],
                        start=(k == 0),
                        stop=(k == KD - 1),
                    )
                lsb = wkp.tile([P, E], f32, tag="lsb", bufs=2, name=f"lsb{g}")
                nc.vector.tensor_tensor(
                    out=lsb[:], in0=lps[:], in1=brb[:], op=AL.add
                )
                negm = wkp.tile([P, 1], f32, tag="negm", bufs=2, name=f"negm{g}")
                nc.vector.tensor_reduce(
                    negm[:], lsb[:], axis=AX.X, op=AL.max, negate=True
                )
                p8 = wkp.tile([P, E], f32, tag="p8", bufs=2, name=f"p8_{g}")
                nc.scalar.activation(
                    p8[:], lsb[:], AF.Exp, bias=negm[:, 0:1], scale=1.0
                )
                s1 = wkp.tile([P, 1], f32, tag="s1", bufs=2, name=f"s1_{g}")
                nc.vector.tensor_reduce(s1[:], p8[:], axis=AX.X, op=AL.add)
                nc.vector.reciprocal(prq[:, g : g + 1], s1[:])

            def g1_block(ni):
                n0, nw = NB[ni]
                for m in range(KH):
                    ps = pp.tile(
                        [P, nw], f32, tag=f"mm{ni}", bufs=2, name=f"g1ps{m}_{ni}"
                    )
                    for k in range(KD):
                        nc.tensor.matmul(
                            ps[:],
                            lhsT=w1s[m][:, k * P : (k + 1) * P],
                            rhs=xts[ni][:, k, 0:nw],
                            start=(k == 0),
                            stop=(k == KD - 1),
                        )
                    nc.scalar.activation(
                        hT[:, m, n0 : n0 + nw], ps[:], AF.Relu,
                        bias=b1_sb[:, m : m + 1], scale=1.0,
                    )

            def g2_block(ni):
                n0, nw = NB[ni]
                for m in range(KD):
                    ps = pp.tile(
                        [P, nw], f32, tag=f"mm{ni}", bufs=2, name=f"g2ps{m}_{ni}"
                    )
                    for k in range(KH):
                        nc.tensor.matmul(
                            ps[:],
                            lhsT=w2s[m][:, k * P : (k + 1) * P],
                            rhs=hT[:, k, n0 : n0 + nw],
                            start=(k == 0),
                            stop=(k == KH - 1),
                        )
                    ytt = wkp.tile(
                        [P, nw], f32, tag=f"ytt{ni}", bufs=2, name=f"ytt{m}_{ni}"
                    )
                    nc.vector.tensor_scalar(
                        out=ytt[:], in0=ps[:],
                        scalar1=b2_sb[:, m : m + 1], scalar2=None, op0=AL.add,
                    )
                    nc.vector.tensor_tensor(
                        out=ytt[:], in0=ytt[:],
                        in1=sbc[:, n0 : n0 + nw], op=AL.mult,
                    )
                    nc.sync.dma_start(
                        yT.ap()[m * P : (m + 1) * P, n0 : n0 + nw], ytt[:]
                    )

            # router tiles that only need xt0, then GEMM1 on block 0 while
            # the rest of x^T / the weights stream in
            ng0 = min(G, 512 // P if NB[0][1] >= 512 else NB[0][1] // P)
            for g in range(ng0):
                router_tile(g)
            g1_block(0)
            for g in range(ng0, G):
                router_tile(g)
            # scale, in slot order: scflat[g*128+p] = prq[p, g]; broadcast to
            # all partitions as [128, GPAD]
            nc.sync.dma_start(
                scflat.opt().rearrange("(g p) -> p g", p=P), prq[:]
            )
            ssb = wkp.tile([1, GPAD], f32, name="ssb")
            nc.sync.dma_start(
                ssb[:], scflat.opt().rearrange("(o c) -> o c", o=1)
            )
            nc.gpsimd.partition_broadcast(sbc[:], ssb[:])

            for ni in range(1, len(NB)):
                g1_block(ni)
            for ni in range(len(NB)):
                g2_block(ni)

    nc.compile()
    return nc


def get_module(cap=CAP):
    key = ("nc", cap)
    if key not in _cache:
        _cache[key] = _build(cap)
    return _cache[key]


def _route(tok, Wr, br):
    """Host-side placement: which tokens go to which expert/core (argmax of
    the router). Only used for sharding; the device recomputes all values."""
    logits = tok @ Wr + br
    e = logits.argmax(-1)
    lists = []
    for c in range(NCORES):
        ids = np.nonzero(e == c)[0].astype(np.int32)
        lists.append(ids)
    return lists


def make_in_maps(x, Wr, br, W1, b1, W2, b2, cap=CAP):
    import ml_dtypes

    wdt = ml_dtypes.bfloat16 if BF16 else np.float32
    tok = np.ascontiguousarray(np.asarray(x, dtype=np.float32).reshape(T, D))
    Wr = np.ascontiguousarray(np.asarray(Wr, dtype=np.float32))
    br_ = np.asarray(br, dtype=np.float32).reshape(E)
    lists = _route(tok, Wr, br_)
    assert max(len(ids) for ids in lists) <= cap
    tokb = tok.astype(wdt)
    NB = _blocks(cap)
    in_maps = []
    for c in range(NCORES):
        w1c = np.asarray(W1[c], dtype=np.float32)  # [D, H]
        w2c = np.asarray(W2[c], dtype=np.float32)  # [H, D]
        # slab layout [m, p, k, q]: lhsT chunk (k, m)[p, q] = W[128k+p, 128m+q]
        w1tc = np.ascontiguousarray(
            w1c.reshape(D // P, P, H // P, P).transpose(2, 1, 0, 3).astype(wdt)
        )
        w2tc = np.ascontiguousarray(
            w2c.reshape(H // P, P, D // P, P).transpose(2, 1, 0, 3).astype(wdt)
        )
        # compact tokens of this expert, padded with token 0; pre-transposed:
        # xtc[p, k, j] = x[token j, 128k+p]
        npad = ((cap + P - 1) // P) * P
        padded = np.zeros(npad, np.int64)
        padded[: len(lists[c])] = lists[c]
        xg = tokb[padded]  # [npad, D]
        xtc = np.ascontiguousarray(
            xg.T.reshape(KD_ := D // P, P, npad).transpose(1, 0, 2)
        )  # [P, KD, npad]
        im = {
            "wr": np.ascontiguousarray(
                Wr.reshape(D // P, P, E).transpose(1, 0, 2)
            ).astype(wdt),
            "brb": np.ascontiguousarray(
                np.broadcast_to(br_[None, :], (P, E)).astype(np.float32)
            ),
            "w1t": w1tc,
            "b1t": np.ascontiguousarray(
                np.asarray(b1[c], dtype=np.float32).reshape(H // P, P).T
            ),
            "w2t": w2tc,
            "b2t": np.ascontiguousarray(
                np.asarray(b2[c], dtype=np.float32).reshape(D // P, P).T
            ),
        }
        for ni, (n0, nw) in enumerate(NB):
            nwp = nw if nw % P == 0 else ((nw + P - 1) // P) * P
            im[f"xt{ni}"] = np.ascontiguousarray(xtc[:, :, n0 : n0 + nwp])
        in_maps.append(im)
    return in_maps, lists


def combine(results, lists, x_shape):
    out = np.zeros((T, D), dtype=np.float32)
    for c in range(NCORES):
        n = len(lists[c])
        yTc = np.asarray(results[c]["yT"])  # [D, cap]
        out[lists[c]] = yTc[:, :n].T
    return out.reshape(x_shape)


def _unwedge_devices_once():
    # best-effort: clear any wedged state on the axon-tunneled NeuronCores
    # left behind by a previous crashed process
    if _cache.get("reset_done"):
        return
    _cache["reset_done"] = True
    try:
        import ctypes
        import jax

        jax.devices()
        lib = ctypes.CDLL("/opt/axon/libaxon_pjrt.so")
        lib.axon_reset.restype = ctypes.c_int64
        lib.axon_reset()
    except Exception:
        pass


def kernel(x, Wr, br, W1, b1, W2, b2):
    from concourse.bass_utils import run_bass_kernel_spmd

    _unwedge_devices_once()
    tok = np.asarray(x, dtype=np.float32).reshape(T, D)
    lists = _route(
        tok,
        np.asarray(Wr, dtype=np.float32),
        np.asarray(br, dtype=np.float32).reshape(E),
    )
    need = max(len(ids) for ids in lists)
    cap = CAP if need <= CAP else ((need + P - 1) // P) * P
    nc = get_module(cap)
    in_maps, lists = make_in_maps(x, Wr, br, W1, b1, W2, b2, cap=cap)
    res = run_bass_kernel_spmd(nc, in_maps, core_ids=list(range(NCORES)))
    return combine(res.results, lists, np.asarray(x).shape)


# revision 4
# speedup vs baseline: 1.2389x; 1.2389x over previous
# Expert-parallel top-1 MoE layer on 8 Trainium2 NeuronCores.
#
# Math (see reference): T=8192 tokens of dim D=1024, router picks top-1 of
# E=8 experts, token goes through that expert's MLP (D->H->D, relu), output
# scaled by the routed softmax prob.
#
# Sharding: one expert per core. The host computes the router argmax once
# (numpy) purely to decide token PLACEMENT (the "all-to-all dispatch" of the
# sharding hint): it gathers each core's tokens, transposes them to x^T and
# casts to bf16 so the device gets direct, partition-major DMA loads (no
# on-device indirect gathers or PE transposes). All VALUE math is on device:
# each core recomputes the router logits on its compacted tokens to get the
# top-1 softmax prob (= 1/sum(exp(l - max)), argmax-free), runs the expert
# MLP as two grouped GEMMs (bf16 operands, fp32 PSUM accumulation, +bias,
# relu), and scales by the prob. The host applies the inverse permutation
# (pure data movement) to assemble the full output.
import sys

sys.path.insert(0, "/opt/trn_rl_repo")

import numpy as np

T, D, H, E = 8192, 1024, 2048, 8
NCORES = 8
P = 128
CAP = 1120  # per-expert token capacity (max group this input: 1115)
BF16 = True

_cache = {}


def _blocks(cap):
    # N blocks of <=512 columns (PSUM bank width)
    nb = []
    n0 = 0
    while n0 < cap:
        w = min(512, cap - n0)
        nb.append((n0, w))
        n0 += w
    return nb


def _build(cap):
    import concourse.bass as bass
    import concourse.mybir as mybir
    import concourse.tile as tile
    from concourse import bacc

    f32 = mybir.dt.float32
    bt = mybir.dt.bfloat16 if BF16 else f32
    AL = mybir.AluOpType
    AF = mybir.ActivationFunctionType
    AX = mybir.AxisListType

    NB = _blocks(cap)
    G = (cap + P - 1) // P  # router tiles (last may be ragged; x is padded)
    GPAD = G * P  # padded token slots for the scale vector
    KD = D // P  # 8   k-chunks of D
    KH = H // P  # 16  k-chunks of H

    nc = bacc.Bacc(
        "TRN2",
        debug=False,
        enable_asserts=False,
        target_bir_lowering=False,
        num_devices=NCORES,
    )

    # x^T arrives pre-transposed/pre-gathered, one dram tensor per N block,
    # padded to a multiple of 128 columns in the last block:
    # xt{i}[p, k, j] = x_bf16[token(n0+j), k*128+p]
    xts_d = []
    for ni, (n0, nw) in enumerate(NB):
        nwp = nw if nw % P == 0 else ((nw + P - 1) // P) * P
        xts_d.append(
            nc.dram_tensor(f"xt{ni}", [P, KD, nwp], bt, kind="ExternalInput")
        )
    wr = nc.dram_tensor("wr", [P, KD, E], bt, kind="ExternalInput")
    brb_d = nc.dram_tensor("brb", [P, E], f32, kind="ExternalInput")
    # weight slabs: [m, p, k, q] so one m-slab is a single contiguous DMA
    w1t = nc.dram_tensor("w1t", [KH, P, KD, P], bt, kind="ExternalInput")
    b1t = nc.dram_tensor("b1t", [P, KH], f32, kind="ExternalInput")
    w2t = nc.dram_tensor("w2t", [KD, P, KH, P], bt, kind="ExternalInput")
    b2t = nc.dram_tensor("b2t", [P, KD], f32, kind="ExternalInput")

    yT = nc.dram_tensor("yT", [D, cap], f32, kind="ExternalOutput")

    with tile.TileContext(nc) as tc:
        with (
            tc.tile_pool(name="const", bufs=1) as cpool,
            tc.tile_pool(name="dram", bufs=1, space="DRAM") as dpool,
            tc.tile_pool(name="psum", bufs=1, space="PSUM") as pp,
            tc.tile_pool(name="main", bufs=1) as mp,
            tc.tile_pool(name="work", bufs=1) as wkp,
        ):
            # The sync DMA queue moves data strictly in emission order, so
            # emit just-in-time: router consts, then x^T block 0, then the
            # first GEMM1 slabs, then the rest interleaved by first use.
            wr_sb = cpool.tile([P, KD, E], bt, name="wr_sb")
            nc.sync.dma_start(wr_sb[:], wr.ap())
            brb = cpool.tile([P, E], f32, name="brb")
            nc.sync.dma_start(brb[:], brb_d.ap())

            xts = []
            for ni, (n0, nw) in enumerate(NB):
                nwp = xts_d[ni].shape[2]
                xsb = mp.tile([P, KD, nwp], bt, tag=f"xt{ni}", name=f"xt{ni}")
                xts.append(xsb)
            nc.sync.dma_start(xts[0][:], xts_d[0].ap())

            b1_sb = cpool.tile([P, KH], f32, name="b1_sb")
            nc.sync.dma_start(b1_sb[:], b1t.ap())

            w1s = []
            for m in range(KH):
                w1sb = cpool.tile([P, D], bt, tag=f"w1s{m}", name=f"w1sb{m}")
                w1s.append(w1sb)
            for m in range(4):
                nc.sync.dma_start(w1s[m][:], w1t.ap()[m])
            if len(NB) > 1:
                nc.sync.dma_start(xts[1][:], xts_d[1].ap())
            for m in range(4, KH):
                nc.sync.dma_start(w1s[m][:], w1t.ap()[m])

            b2_sb = cpool.tile([P, KD], f32, name="b2_sb")
            nc.sync.dma_start(b2_sb[:], b2t.ap())
            for ni in range(2, len(NB)):
                nc.sync.dma_start(xts[ni][:], xts_d[ni].ap())
            w2s = []
            for m in range(KD):
                w2sb = cpool.tile([P, H], bt, tag=f"w2s{m}", name=f"w2sb{m}")
                nc.sync.dma_start(w2sb[:], w2t.ap()[m])
                w2s.append(w2sb)

            # dummy matmuls to trip the PE HAM clock-gate to full speed while
            # the x^T DMA is still in flight
            wjunk = cpool.tile([P, 512], bt, name="wjunk")
            nc.vector.memset(wjunk[:], 0.5)
            wps = pp.tile([P, 512], f32, tag="mm0", bufs=2, name="wps")
            for w in range(10):
                nc.tensor.matmul(
                    wps[:], lhsT=wjunk[:, 0:P], rhs=wjunk[:],
                    start=(w == 0), stop=(w == 9),
                )

            scflat = dpool.tile([GPAD], f32, name="scflat")
            prq = mp.tile([P, G], f32, name="prq")
            sbc = mp.tile([P, GPAD], f32, name="sbc")
            hT = mp.tile([P, KH, cap], bt, name="hT")

            def router_tile(g):
                # top-1 softmax prob of the 128 tokens in tile g
                # (tile g spans columns [g*128, g*128+128) of the padded x^T;
                # 512-multiple N blocks keep tiles inside one block)
                ni = (g * P) // 512
                off = g * P - NB[ni][0]
                lps = pp.tile([P, E], f32, tag="small", bufs=2, name=f"lps{g}")
                for k in range(KD):
                    nc.tensor.matmul(
                        lps[:],
                        lhsT=xts[ni][:, k, off : off + P],
                        rhs=wr_sb[:, k, :],
                        start=(k == 0),
                        stop=(k == KD - 1),
                    )
                lsb = wkp.tile([P, E], f32, tag="lsb", bufs=2, name=f"lsb{g}")
                nc.vector.tensor_tensor(
                    out=lsb[:], in0=lps[:], in1=brb[:], op=AL.add
                )
                negm = wkp.tile([P, 1], f32, tag="negm", bufs=2, name=f"negm{g}")
                nc.vector.tensor_reduce(
                    negm[:], lsb[:], axis=AX.X, op=AL.max, negate=True
                )
                p8 = wkp.tile([P, E], f32, tag="p8", bufs=2, name=f"p8_{g}")
                nc.scalar.activation(
                    p8[:], lsb[:], AF.Exp, bias=negm[:, 0:1], scale=1.0
                )
                s1 = wkp.tile([P, 1], f32, tag="s1", bufs=2, name=f"s1_{g}")
                nc.vector.tensor_reduce(s1[:], p8[:], axis=AX.X, op=AL.add)
                nc.vector.reciprocal(prq[:, g : g + 1], s1[:])

            def g1_block(ni):
                n0, nw = NB[ni]
                for m in range(KH):
                    ps = pp.tile(
                        [P, nw], f32, tag=f"mm{ni}", bufs=2, name=f"g1ps{m}_{ni}"
                    )
                    for k in range(KD):
                        nc.tensor.matmul(
                            ps[:],
                            lhsT=w1s[m][:, k * P : (k + 1) * P],
                            rhs=xts[ni][:, k, 0:nw],
                            start=(k == 0),
                            stop=(k == KD - 1),
                        )
                    nc.scalar.activation(
                        hT[:, m, n0 : n0 + nw], ps[:], AF.Relu,
                        bias=b1_sb[:, m : m + 1], scale=1.0,
                    )

            def g2_block(ni):
                n0, nw = NB[ni]
                for m in range(KD):
                    ps = pp.tile(
                        [P, nw], f32, tag=f"mm{ni}", bufs=2, name=f"g2ps{m}_{ni}"
                    )
                    for k in range(KH):
                        nc.tensor.matmul(
                            ps[:],
                            lhsT=w2s[m][:, k * P : (k + 1) * P],
                            rhs=hT[:, k, n0 : n0 + nw],
                            start=(k == 0),
                            stop=(k == KH - 1),
                        )
                    ytt = wkp.tile(
                        [P, nw], f32, tag=f"ytt{ni}", bufs=4, name=f"ytt{m}_{ni}"
                    )
                    nc.vector.tensor_scalar(
                        out=ytt[:], in0=ps[:],
                        scalar1=b2_sb[:, m : m + 1], scalar2=None, op0=AL.add,
                    )
                    nc.vector.tensor_tensor(
                        out=ytt[:], in0=ytt[:],
                        in1=sbc[:, n0 : n0 + nw], op=AL.mult,
                    )
                    nc.sync.dma_start(
                        yT.ap()[m * P : (m + 1) * P, n0 : n0 + nw], ytt[:]
                    )

            # router tiles that only need xt0, then GEMM1 on block 0 while
            # the rest of x^T / the weights stream in
            ng0 = min(G, 512 // P if NB[0][1] >= 512 else NB[0][1] // P)
            for g in range(ng0):
                router_tile(g)
            g1_block(0)
            for g in range(ng0, G):
                router_tile(g)
            # scale, in slot order: scflat[g*128+p] = prq[p, g]; broadcast to
            # all partitions as [128, GPAD]
            nc.sync.dma_start(
                scflat.opt().rearrange("(g p) -> p g", p=P), prq[:]
            )
            ssb = wkp.tile([1, GPAD], f32, name="ssb")
            nc.sync.dma_start(
                ssb[:], scflat.opt().rearrange("(o c) -> o c", o=1)
            )
            nc.gpsimd.partition_broadcast(sbc[:], ssb[:])

            for ni in range(1, len(NB)):
                g1_block(ni)
            for ni in range(len(NB)):
                g2_block(ni)

    nc.compile()
    return nc


def get_module(cap=CAP):
    key = ("nc", cap)
    if key not in _cache:
        _cache[key] = _build(cap)
    return _cache[key]


def _route(tok, Wr, br):
    """Host-side placement: which tokens go to which expert/core (argmax of
    the router). Only used for sharding; the device recomputes all values."""
    logits = tok @ Wr + br
    e = logits.argmax(-1)
    lists = []
    for c in range(NCORES):
        ids = np.nonzero(e == c)[0].astype(np.int32)
        lists.append(ids)
    return lists


def make_in_maps(x, Wr, br, W1, b1, W2, b2, cap=CAP):
    import ml_dtypes

    wdt = ml_dtypes.bfloat16 if BF16 else np.float32
    tok = np.ascontiguousarray(np.asarray(x, dtype=np.float32).reshape(T, D))
    Wr = np.ascontiguousarray(np.asarray(Wr, dtype=np.float32))
    br_ = np.asarray(br, dtype=np.float32).reshape(E)
    lists = _route(tok, Wr, br_)
    assert max(len(ids) for ids in lists) <= cap
    tokb = tok.astype(wdt)
    NB = _blocks(cap)
    in_maps = []
    for c in range(NCORES):
        w1c = np.asarray(W1[c], dtype=np.float32)  # [D, H]
        w2c = np.asarray(W2[c], dtype=np.float32)  # [H, D]
        # slab layout [m, p, k, q]: lhsT chunk (k, m)[p, q] = W[128k+p, 128m+q]
        w1tc = np.ascontiguousarray(
            w1c.reshape(D // P, P, H // P, P).transpose(2, 1, 0, 3).astype(wdt)
        )
        w2tc = np.ascontiguousarray(
            w2c.reshape(H // P, P, D // P, P).transpose(2, 1, 0, 3).astype(wdt)
        )
        # compact tokens of this expert, padded with token 0; pre-transposed:
        # xtc[p, k, j] = x[token j, 128k+p]
        npad = ((cap + P - 1) // P) * P
        padded = np.zeros(npad, np.int64)
        padded[: len(lists[c])] = lists[c]
        xg = tokb[padded]  # [npad, D]
        xtc = np.ascontiguousarray(
            xg.T.reshape(KD_ := D // P, P, npad).transpose(1, 0, 2)
        )  # [P, KD, npad]
        im = {
            "wr": np.ascontiguousarray(
                Wr.reshape(D // P, P, E).transpose(1, 0, 2)
            ).astype(wdt),
            "brb": np.ascontiguousarray(
                np.broadcast_to(br_[None, :], (P, E)).astype(np.float32)
            ),
            "w1t": w1tc,
            "b1t": np.ascontiguousarray(
                np.asarray(b1[c], dtype=np.float32).reshape(H // P, P).T
            ),
            "w2t": w2tc,
            "b2t": np.ascontiguousarray(
                np.asarray(b2[c], dtype=np.float32).reshape(D // P, P).T
            ),
        }
        for ni, (n0, nw) in enumerate(NB):
            nwp = nw if nw % P == 0 else ((nw + P - 1) // P) * P
            im[f"xt{ni}"] = np.ascontiguousarray(xtc[:, :, n0 : n0 + nwp])
        in_maps.append(im)
    return in_maps, lists


def combine(results, lists, x_shape):
    out = np.zeros((T, D), dtype=np.float32)
    for c in range(NCORES):
        n = len(lists[c])
        yTc = np.asarray(results[c]["yT"])  # [D, cap]
        out[lists[c]] = yTc[:, :n].T
    return out.reshape(x_shape)


def _unwedge_devices_once():
    # best-effort: clear any wedged state on the axon-tunneled NeuronCores
    # left behind by a previous crashed process
    if _cache.get("reset_done"):
        return
    _cache["reset_done"] = True
    try:
        import ctypes
        import jax

        jax.devices()
        lib = ctypes.CDLL("/opt/axon/libaxon_pjrt.so")
        lib.axon_reset.restype = ctypes.c_int64
        lib.axon_reset()
    except Exception:
        pass


def kernel(x, Wr, br, W1, b1, W2, b2):
    from concourse.bass_utils import run_bass_kernel_spmd

    _unwedge_devices_once()
    tok = np.asarray(x, dtype=np.float32).reshape(T, D)
    lists = _route(
        tok,
        np.asarray(Wr, dtype=np.float32),
        np.asarray(br, dtype=np.float32).reshape(E),
    )
    need = max(len(ids) for ids in lists)
    cap = CAP if need <= CAP else ((need + P - 1) // P) * P
    nc = get_module(cap)
    in_maps, lists = make_in_maps(x, Wr, br, W1, b1, W2, b2, cap=cap)
    res = run_bass_kernel_spmd(nc, in_maps, core_ids=list(range(NCORES)))
    return combine(res.results, lists, np.asarray(x).shape)
